# revision 12
# baseline (speedup 1.0000x reference)
"""Trainium2 Bass kernel for ClassificationByRetrieval (segment-max of cosine sims).

Computation: logits[b, c] = max_{n: label[n]==c} <x_b/|x_b|, e_n/|e_n|>
  x: [256, 128], index_embeddings: [200000, 128], labels: [200000], C=1000.

Strategy:
  Host: L2-normalize, sort index rows by class, pad every class to S rows
  (duplicating a real member, which never changes the max), so the segment-max
  becomes a uniform grouped max along contiguous columns. Shard classes across
  the 8 cores (C/8 classes each) -> no cross-core reduction at all.
  Device (per core, raw Bass SPMD program): stream idxT slab [D=128, CPC*S]
  from HBM, matmul with the replicated query block qT [128, 256] -> PSUM sims
  tiles [128 batch, 512], grouped max along the free dim -> logits
  [128, CPC] x 2 batch halves -> HBM.
"""

import os
import sys

import numpy as np

for _p in ("/opt/trn_rl_repo",):
    if _p not in sys.path and os.path.isdir(_p):
        sys.path.append(_p)

B = 256  # queries
D = 128  # embedding dim
NCORES = 8
F = 512  # psum tile free dim (one bank of fp32)

TRACE = False  # set True (e.g. from test.py) to capture an NTFF profile
LAST = None  # last BassKernelResults, for test harness inspection

_BUILD_CACHE = {}


def _build(npc, s, cpc, dt_in_name):
    """Build the per-core Bass program (raw bass, explicit sync).

    npc: columns (padded index rows) per core = cpc * s
    s:   padded class size (rows per class)
    cpc: classes per core
    """
    key = (npc, s, cpc, dt_in_name)
    if key in _BUILD_CACHE:
        return _BUILD_CACHE[key]

    import concourse.bass as bass
    import concourse.mybir as mybir
    from concourse.bass import ds, ts

    dt_in = getattr(mybir.dt, dt_in_name)
    f32 = mybir.dt.float32

    nc = bass.Bass("TRN2", target_bir_lowering=False, debug=False)
    qT = nc.dram_tensor("qT", [D, B], dt_in, kind="ExternalInput").ap()
    idxT = nc.dram_tensor("idxT", [D, npc], dt_in, kind="ExternalInput").ap()
    out = nc.dram_tensor("out", [B, cpc], f32, kind="ExternalOutput").ap()

    n_full, tail = divmod(npc, F)
    assert tail % s == 0
    g_full = F // s
    ntiles = n_full + (1 if tail else 0)
    NPS = 8  # psum slots (banks)
    TB = 8  # psum tiles per DMA batch (8*512*2B = 1 MiB per transfer)
    XB = 2  # x batch buffers

    # batches of psum tiles: list of (first_tile, n_tiles)
    batches = []
    t0 = 0
    while t0 < ntiles:
        nt = min(TB, ntiles - t0)
        batches.append((t0, nt))
        t0 += nt

    def tile_cols(t):
        return F if t < n_full else tail

    def border(t):
        return (0, 1) if t % 2 == 0 else (1, 0)

    NEG = -3.4e38
    CPB = 4  # ACT copy staging buffers
    A_FRAC = False  # ACT-assisted reduce path (v3)

    # static schedule of psum tiles: (k, t, b, f, g) in issue order
    sched = []
    k = 0
    for t in range(ntiles):
        f = tile_cols(t)
        for b in border(t):
            sched.append((k, t, b, f, f // s))
            k += 1
    nk = len(sched)

    # mode per psum tile: True -> ACT copies to SBUF, DVE reduces from SBUF
    # (TTR, 2 reads/cycle); False -> DVE reduces straight from PSUM.
    # ~3/4 A-tiles balances ACT (copy) against DVE (TTR+reduce). Tail
    # (partial) tiles stay on the direct path for simplicity.
    def is_a(kk):
        return A_FRAC and (kk % 4 != 3) and sched[kk][3] == F

    a_index = {}  # k -> running A-tile ordinal (0-based)
    na = 0
    for kk in range(nk):
        if is_a(kk):
            a_index[kk] = na
            na += 1
    nv = nk - na
    # cumulative counts up to and including k
    cum_a = [0] * nk
    cum_v = [0] * nk
    ca = cv = 0
    for kk in range(nk):
        if is_a(kk):
            ca += 1
        else:
            cv += 1
        cum_a[kk] = ca
        cum_v[kk] = cv

    from contextlib import ExitStack

    with ExitStack() as ctx:
        q_sb = ctx.enter_context(nc.sbuf_tensor([D, B], dt_in))
        x_sb = ctx.enter_context(nc.sbuf_tensor([D, XB, TB * F], dt_in))
        log_sb = ctx.enter_context(nc.sbuf_tensor([128, 2, cpc], f32))
        cp_sb = ctx.enter_context(nc.sbuf_tensor([128, CPB, F], f32))
        tt_scr = ctx.enter_context(nc.sbuf_tensor([128, s // 2], f32))
        ps = ctx.enter_context(nc.psum_tensor([128, NPS, F], f32))
        sem_q = ctx.enter_context(nc.semaphore())
        # one sem per x buffer slot: a slot has at most one DMA in flight,
        # so cumulative per-slot waits are unambiguous
        sem_x = [
            ctx.enter_context(nc.semaphore(name=f"sem_x{i}")) for i in range(XB)
        ]
        sem_mm = ctx.enter_context(nc.semaphore())  # PE matmuls done
        sem_cp = ctx.enter_context(nc.semaphore())  # ACT psum->sbuf copies done
        sem_fv = ctx.enter_context(nc.semaphore())  # V-tile psum freed (DVE)
        sem_tt = ctx.enter_context(nc.semaphore())  # A-tile TTRs done (DVE)
        sem_out = ctx.enter_context(nc.semaphore())
        block = ctx.enter_context(nc.Block())

        @block.sync
        def _(sp):
            sp.dma_start(q_sb[:, :], qT).then_inc(sem_q, 16)
            for bi, (bt, nt) in enumerate(batches):
                cols = sum(tile_cols(bt + i) for i in range(nt))
                if bi >= XB:
                    # x slot reuse: all matmuls of batch bi-XB must be done
                    pt, pn = batches[bi - XB]
                    sp.wait_ge(sem_mm, 2 * (pt + pn))
                sp.dma_start(
                    x_sb[:, bi % XB, :cols], idxT[:, ds(bt * F, cols)]
                ).then_inc(sem_x[bi % XB], 16)
            sp.wait_ge(sem_fv, nv)
            sp.wait_ge(sem_tt, 2 * na)
            sp.dma_start(out[ts(0, 128), :], log_sb[:, 0, :]).then_inc(sem_out, 16)
            sp.dma_start(out[ts(1, 128), :], log_sb[:, 1, :]).then_inc(sem_out, 16)
            sp.wait_ge(sem_out, 32)

        @block.tensor
        def _(pe):
            pe.wait_ge(sem_q, 16)
            for kk, t, b, f, g in sched:
                if kk % (2 * TB) == 0:
                    bi = t // TB
                    pe.wait_ge(sem_x[bi % XB], 16 * (bi // XB + 1))
                if kk >= NPS:
                    # psum slot reuse: tile kk-NPS must be drained by its
                    # mode's engine
                    j = kk - NPS
                    if is_a(j):
                        pe.wait_ge(sem_cp, cum_a[j])
                    else:
                        pe.wait_ge(sem_fv, cum_v[j])
                bi = t // TB
                nc.tensor.matmul(
                    ps[:, kk % NPS, :f],
                    lhsT=q_sb[:, ts(b, 128)],
                    rhs=x_sb[:, bi % XB, ds((t - bi * TB) * F, f)],
                    start=True,
                    stop=True,
                ).then_inc(sem_mm, 1)

        @block.scalar
        def _(act):
            for kk, t, b, f, g in sched:
                if not is_a(kk):
                    continue
                ai = a_index[kk]
                if ai >= CPB:
                    # staging slot reuse: TTRs of the A-tile CPB back done
                    act.wait_ge(sem_tt, 2 * (ai - CPB + 1))
                act.wait_ge(sem_mm, kk + 1)
                nc.scalar.copy(
                    cp_sb[:, ai % CPB, :f], ps[:, kk % NPS, :f]
                ).then_inc(sem_cp, 1)

        @block.vector
        def _(ve):
            h = s // 2
            for kk, t, b, f, g in sched:
                if is_a(kk):
                    ai = a_index[kk]
                    ve.wait_ge(sem_cp, ai + 1)
                    for c in range(g):
                        # class max from SBUF staging: fused elementwise max
                        # of the two class halves + reduction (2 reads/cycle)
                        nc.vector.tensor_tensor_reduce(
                            out=tt_scr[:, :h],
                            in0=cp_sb[:, ai % CPB, ds(c * s, h)],
                            in1=cp_sb[:, ai % CPB, ds(c * s + h, h)],
                            scale=1.0,
                            scalar=NEG,
                            op0=mybir.AluOpType.max,
                            op1=mybir.AluOpType.max,
                            accum_out=log_sb[:, b, ds(t * g_full + c, 1)],
                        ).then_inc(sem_tt, 1)
                else:
                    ve.wait_ge(sem_mm, kk + 1)
                    nc.vector.reduce_max(
                        log_sb[:, b, ds(t * g_full, g)],
                        ps[:, kk % NPS, :f].rearrange("p (g s) -> p g s", s=s),
                        mybir.AxisListType.X,
                    ).then_inc(sem_fv, 1)

    _BUILD_CACHE[key] = nc
    return nc


def _prep(x, index_embeddings, class_labels, num_classes):
    """Host-side layout: normalize, group-by-class, pad, shard. Returns
    (qT, per-core slabs, counts, C, s, cpc, npc, dt_np)."""
    C = int(num_classes)
    x = np.asarray(x, dtype=np.float32)
    idx = np.asarray(index_embeddings, dtype=np.float32)
    labels = np.asarray(class_labels).astype(np.int64)
    N = idx.shape[0]

    qn = x / np.maximum(np.sqrt((x * x).sum(-1, keepdims=True)), 1e-12)
    en = idx / np.maximum(np.sqrt((idx * idx).sum(-1, keepdims=True)), 1e-12)

    # pad class count to a multiple of NCORES with empty classes
    C_pad = ((C + NCORES - 1) // NCORES) * NCORES
    counts = np.bincount(labels, minlength=C_pad)
    order = np.argsort(labels, kind="stable")
    starts = np.zeros(C_pad, dtype=np.int64)
    np.cumsum(counts[:-1], out=starts[1:])

    s = 256
    while s < counts.max():
        s += 256

    # perm[c, j] = index row for slot j of class c (pad with last member)
    j = np.minimum(np.arange(s)[None, :], np.maximum(counts - 1, 0)[:, None])
    perm = order[np.minimum(starts[:, None] + j, N - 1)]  # [C_pad, s]

    dt_np = np.float16
    idx_pad = en[perm.reshape(-1)]  # [C_pad * s, D]
    qT = np.ascontiguousarray(qn.T.astype(dt_np))  # [D, B]

    cpc = C_pad // NCORES
    npc = cpc * s
    slabs = [
        np.ascontiguousarray(idx_pad[k * npc : (k + 1) * npc].T.astype(dt_np))
        for k in range(NCORES)
    ]
    return qT, slabs, counts, C, s, cpc, npc, dt_np


def kernel(x, index_embeddings, class_labels, num_classes):
    from concourse import bass_utils

    global LAST

    qT, slabs, counts, C, s, cpc, npc, dt_np = _prep(
        x, index_embeddings, class_labels, num_classes
    )
    dt_name = {np.float32: "float32", np.float16: "float16"}[dt_np]
    nc = _build(npc, s, cpc, dt_name)

    in_maps = [{"qT": qT, "idxT": slab} for slab in slabs]
    res = bass_utils.run_bass_kernel_spmd(
        nc,
        in_maps,
        core_ids=list(range(NCORES)),
        trace=TRACE,
        trace_cores=list(range(NCORES)) if TRACE else None,
    )
    LAST = res

    logits = np.concatenate([res.results[k]["out"] for k in range(NCORES)], axis=1)
    logits = logits[:, :C].astype(np.float32)
    logits[:, counts[:C] == 0] = -np.inf
    return logits
